# revision 6
# baseline (speedup 1.0000x reference)
"""
Trainium2 Bass kernel for nn_CausalMatrixGameTransformerBlock (streaming-window
attention).

Math (shapes hardcoded from the problem spec):
  B=1, S=1920 new tokens, N=12 heads, D=128, CACHE=6720,
  f=2, h=24, w=40, current_start=global_end=local_end=5760.

  With those static ints the reference reduces to:
    rq = rope(q), rk = rope(k)
    K = concat(cache_k[:, 1920:5760], rk)   # [5760, 12, 128] window per head
    V = concat(cache_v[:, 1920:5760], v)
    out[q,n,:] = softmax(rq K^T / sqrt(128)) V    dense over 5760 keys.

Sharding: 24 units of (head, 960-query-half) -> each core gets 3 units that
cover exactly one FULL head (1920 q) plus one HALF head (960 q), so each core
loads K/V for only 2 heads.  SPMD: one program, per-core input data; per-core
q layout is [full-head 1920 | half-head 960] so the program is uniform.

Host does all elementwise prep: RoPE(q), RoPE(new k), window concat,
transposes, fp16 casts.  Device does the attention proper:

  per 480-query chunk (6 per core), kk tiles of 128 in groups of 3:
     ps[b]   = K_t^T @ q_chunk      (PE, fp16 -> f32 PSUM, 6-bank region)
     ex      = exp(ps * 1/sqrt(D))  (ACT, one instr per 3-tile group, fp16 out)
     exacc  += ex[:, i, :]          (DVE fp16 adds -> per-lane denominator)
     po     += V_t^T @ ex[:, i, :]  (PE, accumulating f32 PSUM)
  DMA po (f32) and exacc (fp16) to DRAM; host computes
  out = poT / sum_lanes(exacc) and scatters.  No on-device softmax divide and
  no ones-matmul denominator pass.
"""

import numpy as np

N_CORES = 8
S = 1920
NHEADS = 12
D = 128
WIN = 5760           # attention window (keys)
KTILES = WIN // 128  # 45
QCHUNK = 480
NCHUNK = 6           # 2880 queries per core / 480
SCALE = 1.0 / np.sqrt(np.float64(D))

_PROG = None


def _rope_tables():
    """cos/sin angle tables [1920, 64] exactly as the reference builds them."""
    def rope_angles(max_len, dim, theta=10000.0):
        inv = 1.0 / (theta ** (np.arange(0, dim, 2, dtype=np.float64) / dim))
        return np.outer(np.arange(max_len, dtype=np.float64), inv)

    d = D
    freqs = np.concatenate([
        rope_angles(1024, d - 4 * (d // 6)),
        rope_angles(1024, 2 * (d // 6)),
        rope_angles(1024, 2 * (d // 6)),
    ], axis=1).astype(np.float32)          # [1024, 64]

    f, h, w = 2, 24, 40
    start_frame = 6                         # current_start // (h*w)
    c = d // 2
    s0, s1 = c - 2 * (c // 3), c // 3       # 22, 21
    ang = np.concatenate([
        np.broadcast_to(freqs[start_frame:start_frame + f, :s0][:, None, None, :], (f, h, w, s0)),
        np.broadcast_to(freqs[:h, s0:s0 + s1][None, :, None, :], (f, h, w, s1)),
        np.broadcast_to(freqs[:w, s0 + s1:][None, None, :, :], (f, h, w, s1)),
    ], axis=-1).reshape(S, c)
    return np.cos(ang).astype(np.float32), np.sin(ang).astype(np.float32)


def _rope(x, cos, sin):
    """x: [S, N, D] f32; cos/sin: [S, 64]. Interleaved-pair rotation."""
    x0, x1 = x[..., 0::2], x[..., 1::2]
    c, s = cos[:, None, :], sin[:, None, :]
    out = np.empty_like(x)
    out[..., 0::2] = x0 * c - x1 * s
    out[..., 1::2] = x0 * s + x1 * c
    return out


def _segs_for_core(c):
    """Returns ((full_head, 0, 1920), (half_head, q0, 960)) for core c."""
    if c % 2 == 0:
        return (3 * c // 2, 0, 1920), ((3 * c + 2) // 2, 0, 960)
    return ((3 * c + 2) // 2, 0, 1920), (3 * c // 2, 960, 960)


def _build_program():
    from contextlib import ExitStack
    from concourse import bacc
    import concourse.tile as tile
    import concourse.mybir as mybir

    F32 = mybir.dt.float32
    F16 = mybir.dt.float16
    EXP = mybir.ActivationFunctionType.Exp

    nc = bacc.Bacc("TRN2", target_bir_lowering=False, debug=False,
                   enable_asserts=False, num_devices=N_CORES)

    rqT = nc.dram_tensor("rqT", [128, 6 * QCHUNK], F16, kind="ExternalInput").ap()
    kT = nc.dram_tensor("kT", [2, 128, WIN], F16, kind="ExternalInput").ap()
    vin = nc.dram_tensor("vin", [2, 128, KTILES, 128], F16, kind="ExternalInput").ap()
    poT = nc.dram_tensor("poT", [NCHUNK, 128, QCHUNK], F32, kind="ExternalOutput").ap()
    exo = nc.dram_tensor("exo", [NCHUNK, 128, QCHUNK], F16, kind="ExternalOutput").ap()

    with ExitStack() as ctx:
        tc = ctx.enter_context(tile.TileContext(nc))
        inp = ctx.enter_context(tc.tile_pool(name="inp", bufs=1))
        expp = ctx.enter_context(tc.tile_pool(name="ex", bufs=4))
        accp = ctx.enter_context(tc.tile_pool(name="acc", bufs=2))
        outp = ctx.enter_context(tc.tile_pool(name="outp", bufs=2))
        pss = ctx.enter_context(tc.tile_pool(name="pss", bufs=1, space="PSUM"))
        pso = ctx.enter_context(tc.tile_pool(name="pso", bufs=2, space="PSUM"))

        q_sb = inp.tile([128, 6 * QCHUNK], F16, name="q_sb")
        k_sb = [inp.tile([128, WIN], F16, name=f"k_sb{h}") for h in range(2)]
        v_sb = [inp.tile([128, KTILES, 128], F16, name=f"v_sb{h}") for h in range(2)]
        ps = pss.tile([128, 6, 512], F32, name="ps")   # 6 PSUM banks, manual rotation

        # ---- input DMAs: first chunk's q + head0's first K piece land first ----
        nc.sync.dma_start(out=q_sb[:, 0:QCHUNK], in_=rqT[:, 0:QCHUNK])
        KP, VP = 4, 3    # pieces per K / V load
        for h in range(2):
            for p in range(KP):
                w0, w1 = p * (WIN // KP), (p + 1) * (WIN // KP)
                nc.sync.dma_start(out=k_sb[h][:, w0:w1], in_=kT[h, :, w0:w1])
                if h == 0 and p == 0:
                    nc.sync.dma_start(out=q_sb[:, QCHUNK:], in_=rqT[:, QCHUNK:])
                if p < VP:
                    t0, t1 = p * (KTILES // VP), (p + 1) * (KTILES // VP)
                    nc.sync.dma_start(out=v_sb[h][:, t0:t1, :], in_=vin[h, :, t0:t1, :])

        NG = KTILES // 3   # 15 groups of 3 kk-tiles
        for c in range(NCHUNK):
            hsel = 0 if c < 4 else 1
            ksb, vsb = k_sb[hsel], v_sb[hsel]
            qs = q_sb[:, c * QCHUNK:(c + 1) * QCHUNK]
            po = pso.tile([128, QCHUNK], F32, name="po")
            exacc = accp.tile([128, QCHUNK], F16, name="exacc")

            def _pv(pex, pg):
                for i in range(3):
                    t = 3 * pg + i
                    nc.tensor.matmul(out=po, lhsT=vsb[:, t, :], rhs=pex[:, i, :],
                                     start=(t == 0), stop=(t == KTILES - 1))

            # two-group PE lookahead: in-order PE must finish QK(g+1) before it
            # blocks on exp-dependent PV(g-1), or ACT starves for a full group
            pend = []     # [(ex, g)] pending PV groups, depth 2
            for g in range(NG):
                half = (g + c) % 2          # 15 groups/chunk -> halves alternate
                for i in range(3):
                    t = 3 * g + i
                    nc.tensor.matmul(out=ps[:, 3 * half + i, 0:QCHUNK],
                                     lhsT=ksb[:, t * 128:(t + 1) * 128],
                                     rhs=qs, start=True, stop=True)
                ex = expp.tile([128, 3, QCHUNK], F16, name="ex")
                nc.scalar.activation(out=ex, in_=ps[:, 3 * half:3 * half + 3, 0:QCHUNK],
                                     func=EXP, scale=float(SCALE))
                for i in range(3):
                    if g == 0 and i == 0:
                        # seed the accumulator (avoids a memset pass)
                        nc.vector.tensor_add(exacc, ex[:, 0, :], ex[:, 1, :])
                    elif g == 0 and i == 1:
                        pass
                    else:
                        nc.vector.tensor_add(exacc, exacc, ex[:, i, :])
                pend.append((ex, g))
                if len(pend) > 2:
                    _pv(*pend.pop(0))
            for ent in pend:
                _pv(*ent)
            po_sb = outp.tile([128, QCHUNK], F32, name="po_sb")
            nc.vector.tensor_copy(out=po_sb, in_=po)   # PSUM->SBUF eviction
            nc.sync.dma_start(out=poT[c], in_=po_sb)
            nc.sync.dma_start(out=exo[c], in_=exacc)

    nc.compile()
    return nc


def _get_program():
    global _PROG
    if _PROG is None:
        _PROG = _build_program()
    return _PROG


def _host_prep(q, k, v, cache_k, cache_v):
    """Build the 8 per-core input maps (rope + window concat + fp16 on host)."""
    cos, sin = _rope_tables()
    rq = _rope(np.asarray(q, np.float32)[0], cos, sin)      # [1920, 12, 128]
    rk = _rope(np.asarray(k, np.float32)[0], cos, sin)
    Kfull = np.concatenate([np.asarray(cache_k, np.float32)[0, 1920:5760], rk], axis=0)
    Vfull = np.concatenate([np.asarray(cache_v, np.float32)[0, 1920:5760],
                            np.asarray(v, np.float32)[0]], axis=0)  # [5760, 12, 128]
    rq16 = rq.astype(np.float16)
    K16 = Kfull.astype(np.float16)
    V16 = Vfull.astype(np.float16)

    in_maps = []
    for c in range(N_CORES):
        (fh, _, _), (hh, hq0, _) = _segs_for_core(c)
        qcat = np.concatenate([rq16[:, fh, :], rq16[hq0:hq0 + 960, hh, :]], axis=0)
        rqT = np.ascontiguousarray(qcat.T)                  # [128, 2880]
        kTa = np.stack([np.ascontiguousarray(K16[:, h, :].T) for h in (fh, hh)])
        va = np.stack([np.ascontiguousarray(
            V16[:, h, :].reshape(KTILES, 128, 128).transpose(1, 0, 2)) for h in (fh, hh)])
        in_maps.append({"rqT": rqT, "kT": kTa, "vin": va})
    return in_maps


def _gather(results):
    out = np.empty((1, S, NHEADS, D), np.float32)
    for c in range(N_CORES):
        poT = results[c]["poT"]                             # [6, 128, 480] f32
        exo = results[c]["exo"].astype(np.float32)          # [6, 128, 480]
        den = exo.sum(axis=1)                               # [6, 480]
        o = poT / den[:, None, :]                           # [6, 128, 480]
        (fh, _, _), (hh, hq0, _) = _segs_for_core(c)
        full = o[0:4].transpose(0, 2, 1).reshape(1920, 128)
        half = o[4:6].transpose(0, 2, 1).reshape(960, 128)
        out[0, :, fh, :] = full
        out[0, hq0:hq0 + 960, hh, :] = half
    return out


def kernel(q, k, v, cache_k, cache_v, f=2, h=24, w=40,
           current_start=5760, global_end=5760, local_end=5760, **_extra):
    from concourse.bass_utils import run_bass_kernel_spmd

    nc = _get_program()
    in_maps = _host_prep(q, k, v, cache_k, cache_v)
    res = run_bass_kernel_spmd(nc, in_maps, list(range(N_CORES)))
    return _gather(res.results)


# revision 8
# speedup vs baseline: 1.6270x; 1.6270x over previous
"""
Trainium2 Bass kernel for nn_CausalMatrixGameTransformerBlock (streaming-window
attention).

Math (shapes hardcoded from the problem spec):
  B=1, S=1920 new tokens, N=12 heads, D=128, CACHE=6720,
  f=2, h=24, w=40, current_start=global_end=local_end=5760.

  With those static ints the reference reduces to:
    rq = rope(q), rk = rope(k)
    K = concat(cache_k[:, 1920:5760], rk)   # [5760, 12, 128] window per head
    V = concat(cache_v[:, 1920:5760], v)
    out[q,n,:] = softmax(rq K^T / sqrt(128)) V    dense over 5760 keys.

Sharding: 24 units of (head, 960-query-half) -> each core gets 3 units that
cover exactly one FULL head (1920 q) plus one HALF head (960 q), so each core
loads K/V for only 2 heads.  SPMD: one program, per-core input data; per-core
q layout is [full-head 1920 | half-head 960] so the program is uniform.

Host does all elementwise prep: RoPE(q), RoPE(new k), window concat,
transposes, fp16 casts.  Device does the attention proper:

  per 480-query chunk (6 per core), kk tiles of 128 in groups of 3:
     ps[b]   = K_t^T @ q_chunk      (PE, fp16 -> f32 PSUM, 6-bank region)
     ex      = exp(ps * 1/sqrt(D))  (ACT, one instr per 3-tile group, fp16 out)
     exacc  += ex[:, i, :]          (DVE fp16 adds -> per-lane denominator)
     po     += V_t^T @ ex[:, i, :]  (PE, accumulating f32 PSUM)
  DMA po (f32) and exacc (fp16) to DRAM; host computes
  out = poT / sum_lanes(exacc) and scatters.  No on-device softmax divide and
  no ones-matmul denominator pass.
"""

import numpy as np

N_CORES = 8
S = 1920
NHEADS = 12
D = 128
WIN = 5760           # attention window (keys)
KTILES = WIN // 128  # 45
QCHUNK = 480
NCHUNK = 6           # 2880 queries per core / 480
SCALE = 1.0 / np.sqrt(np.float64(D))

_PROG = None


def _rope_tables():
    """cos/sin angle tables [1920, 64] exactly as the reference builds them."""
    def rope_angles(max_len, dim, theta=10000.0):
        inv = 1.0 / (theta ** (np.arange(0, dim, 2, dtype=np.float64) / dim))
        return np.outer(np.arange(max_len, dtype=np.float64), inv)

    d = D
    freqs = np.concatenate([
        rope_angles(1024, d - 4 * (d // 6)),
        rope_angles(1024, 2 * (d // 6)),
        rope_angles(1024, 2 * (d // 6)),
    ], axis=1).astype(np.float32)          # [1024, 64]

    f, h, w = 2, 24, 40
    start_frame = 6                         # current_start // (h*w)
    c = d // 2
    s0, s1 = c - 2 * (c // 3), c // 3       # 22, 21
    ang = np.concatenate([
        np.broadcast_to(freqs[start_frame:start_frame + f, :s0][:, None, None, :], (f, h, w, s0)),
        np.broadcast_to(freqs[:h, s0:s0 + s1][None, :, None, :], (f, h, w, s1)),
        np.broadcast_to(freqs[:w, s0 + s1:][None, None, :, :], (f, h, w, s1)),
    ], axis=-1).reshape(S, c)
    return np.cos(ang).astype(np.float32), np.sin(ang).astype(np.float32)


def _rope(x, cos, sin):
    """x: [S, N, D] f32; cos/sin: [S, 64]. Interleaved-pair rotation."""
    x0, x1 = x[..., 0::2], x[..., 1::2]
    c, s = cos[:, None, :], sin[:, None, :]
    out = np.empty_like(x)
    out[..., 0::2] = x0 * c - x1 * s
    out[..., 1::2] = x0 * s + x1 * c
    return out


def _segs_for_core(c):
    """Returns ((full_head, 0, 1920), (half_head, q0, 960)) for core c."""
    if c % 2 == 0:
        return (3 * c // 2, 0, 1920), ((3 * c + 2) // 2, 0, 960)
    return ((3 * c + 2) // 2, 0, 1920), (3 * c // 2, 960, 960)


def _build_program():
    from contextlib import ExitStack
    from concourse import bacc
    import concourse.tile as tile
    import concourse.mybir as mybir

    F32 = mybir.dt.float32
    F16 = mybir.dt.float16
    EXP = mybir.ActivationFunctionType.Exp

    nc = bacc.Bacc("TRN2", target_bir_lowering=False, debug=False,
                   enable_asserts=False, num_devices=N_CORES)

    rqT = nc.dram_tensor("rqT", [128, 6 * QCHUNK], F16, kind="ExternalInput").ap()
    kT = nc.dram_tensor("kT", [2, 128, WIN], F16, kind="ExternalInput").ap()
    vin = nc.dram_tensor("vin", [2, 128, KTILES, 128], F16, kind="ExternalInput").ap()
    poT = nc.dram_tensor("poT", [NCHUNK, 128, QCHUNK], F32, kind="ExternalOutput").ap()
    exo = nc.dram_tensor("exo", [NCHUNK, 128, QCHUNK], F16, kind="ExternalOutput").ap()

    with ExitStack() as ctx:
        tc = ctx.enter_context(tile.TileContext(nc))
        inp = ctx.enter_context(tc.tile_pool(name="inp", bufs=1))
        expp = ctx.enter_context(tc.tile_pool(name="ex", bufs=4))
        accp = ctx.enter_context(tc.tile_pool(name="acc", bufs=2))
        outp = ctx.enter_context(tc.tile_pool(name="outp", bufs=2))
        pss = ctx.enter_context(tc.tile_pool(name="pss", bufs=1, space="PSUM"))
        pso = ctx.enter_context(tc.tile_pool(name="pso", bufs=2, space="PSUM"))

        q_sb = inp.tile([128, 6 * QCHUNK], F16, name="q_sb")
        k_sb = [inp.tile([128, WIN], F16, name=f"k_sb{h}") for h in range(2)]
        v_sb = [inp.tile([128, KTILES, 128], F16, name=f"v_sb{h}") for h in range(2)]
        # two 3-bank half-regions; separate tiles so the dep tracker sees the
        # ping-pong (one big tile serializes QK(g+1) behind exp(g))
        ps_h = [pss.tile([128, 3, 512], F32, name=f"ps{h}") for h in range(2)]

        # ---- input DMAs: first chunk's q + head0's first K piece land first ----
        nc.sync.dma_start(out=q_sb[:, 0:QCHUNK], in_=rqT[:, 0:QCHUNK])
        KP, VP = 4, 3    # pieces per K / V load
        for h in range(2):
            for p in range(KP):
                w0, w1 = p * (WIN // KP), (p + 1) * (WIN // KP)
                nc.sync.dma_start(out=k_sb[h][:, w0:w1], in_=kT[h, :, w0:w1])
                if h == 0 and p == 0:
                    nc.sync.dma_start(out=q_sb[:, QCHUNK:], in_=rqT[:, QCHUNK:])
                if p < VP:
                    t0, t1 = p * (KTILES // VP), (p + 1) * (KTILES // VP)
                    nc.sync.dma_start(out=v_sb[h][:, t0:t1, :], in_=vin[h, :, t0:t1, :])

        NG = KTILES // 3   # 15 groups of 3 kk-tiles
        for c in range(NCHUNK):
            hsel = 0 if c < 4 else 1
            ksb, vsb = k_sb[hsel], v_sb[hsel]
            qs = q_sb[:, c * QCHUNK:(c + 1) * QCHUNK]
            po = pso.tile([128, QCHUNK], F32, name="po")
            exacc = accp.tile([128, QCHUNK], F16, name="exacc")

            def _pv(pex, pg):
                for i in range(3):
                    t = 3 * pg + i
                    nc.tensor.matmul(out=po, lhsT=vsb[:, t, :], rhs=pex[:, i, :],
                                     start=(t == 0), stop=(t == KTILES - 1))

            # two-group PE lookahead: in-order PE must finish QK(g+1) before it
            # blocks on exp-dependent PV(g-1), or ACT starves for a full group
            pend = []     # [(ex, g)] pending PV groups, depth 2
            for g in range(NG):
                ps = ps_h[(g + c) % 2]      # 15 groups/chunk -> halves alternate
                for i in range(3):
                    t = 3 * g + i
                    nc.tensor.matmul(out=ps[:, i, 0:QCHUNK],
                                     lhsT=ksb[:, t * 128:(t + 1) * 128],
                                     rhs=qs, start=True, stop=True)
                ex = expp.tile([128, 3, QCHUNK], F16, name="ex")
                nc.scalar.activation(out=ex, in_=ps[:, 0:3, 0:QCHUNK],
                                     func=EXP, scale=float(SCALE))
                for i in range(3):
                    if g == 0 and i == 0:
                        # seed the accumulator (avoids a memset pass)
                        nc.vector.tensor_add(exacc, ex[:, 0, :], ex[:, 1, :])
                    elif g == 0 and i == 1:
                        pass
                    else:
                        nc.vector.tensor_add(exacc, exacc, ex[:, i, :])
                pend.append((ex, g))
                if len(pend) > 2:
                    _pv(*pend.pop(0))
            for ent in pend:
                _pv(*ent)
            po_sb = outp.tile([128, QCHUNK], F32, name="po_sb")
            nc.vector.tensor_copy(out=po_sb, in_=po)   # PSUM->SBUF eviction
            nc.sync.dma_start(out=poT[c], in_=po_sb)
            nc.sync.dma_start(out=exo[c], in_=exacc)

    nc.compile()
    return nc


def _get_program():
    global _PROG
    if _PROG is None:
        _PROG = _build_program()
    return _PROG


def _host_prep(q, k, v, cache_k, cache_v):
    """Build the 8 per-core input maps (rope + window concat + fp16 on host)."""
    cos, sin = _rope_tables()
    rq = _rope(np.asarray(q, np.float32)[0], cos, sin)      # [1920, 12, 128]
    rk = _rope(np.asarray(k, np.float32)[0], cos, sin)
    Kfull = np.concatenate([np.asarray(cache_k, np.float32)[0, 1920:5760], rk], axis=0)
    Vfull = np.concatenate([np.asarray(cache_v, np.float32)[0, 1920:5760],
                            np.asarray(v, np.float32)[0]], axis=0)  # [5760, 12, 128]
    rq16 = rq.astype(np.float16)
    K16 = Kfull.astype(np.float16)
    V16 = Vfull.astype(np.float16)

    in_maps = []
    for c in range(N_CORES):
        (fh, _, _), (hh, hq0, _) = _segs_for_core(c)
        qcat = np.concatenate([rq16[:, fh, :], rq16[hq0:hq0 + 960, hh, :]], axis=0)
        rqT = np.ascontiguousarray(qcat.T)                  # [128, 2880]
        kTa = np.stack([np.ascontiguousarray(K16[:, h, :].T) for h in (fh, hh)])
        va = np.stack([np.ascontiguousarray(
            V16[:, h, :].reshape(KTILES, 128, 128).transpose(1, 0, 2)) for h in (fh, hh)])
        in_maps.append({"rqT": rqT, "kT": kTa, "vin": va})
    return in_maps


def _gather(results):
    out = np.empty((1, S, NHEADS, D), np.float32)
    for c in range(N_CORES):
        poT = results[c]["poT"]                             # [6, 128, 480] f32
        exo = results[c]["exo"].astype(np.float32)          # [6, 128, 480]
        den = exo.sum(axis=1)                               # [6, 480]
        o = poT / den[:, None, :]                           # [6, 128, 480]
        (fh, _, _), (hh, hq0, _) = _segs_for_core(c)
        full = o[0:4].transpose(0, 2, 1).reshape(1920, 128)
        half = o[4:6].transpose(0, 2, 1).reshape(960, 128)
        out[0, :, fh, :] = full
        out[0, hq0:hq0 + 960, hh, :] = half
    return out


def kernel(q, k, v, cache_k, cache_v, f=2, h=24, w=40,
           current_start=5760, global_end=5760, local_end=5760, **_extra):
    from concourse.bass_utils import run_bass_kernel_spmd

    nc = _get_program()
    in_maps = _host_prep(q, k, v, cache_k, cache_v)
    res = run_bass_kernel_spmd(nc, in_maps, list(range(N_CORES)))
    return _gather(res.results)


# revision 9
# speedup vs baseline: 1.6531x; 1.0160x over previous
"""
Trainium2 Bass kernel for nn_CausalMatrixGameTransformerBlock (streaming-window
attention).

Math (shapes hardcoded from the problem spec):
  B=1, S=1920 new tokens, N=12 heads, D=128, CACHE=6720,
  f=2, h=24, w=40, current_start=global_end=local_end=5760.

  With those static ints the reference reduces to:
    rq = rope(q), rk = rope(k)
    K = concat(cache_k[:, 1920:5760], rk)   # [5760, 12, 128] window per head
    V = concat(cache_v[:, 1920:5760], v)
    out[q,n,:] = softmax(rq K^T / sqrt(128)) V    dense over 5760 keys.

Sharding: 24 units of (head, 960-query-half) -> each core gets 3 units that
cover exactly one FULL head (1920 q) plus one HALF head (960 q), so each core
loads K/V for only 2 heads.  SPMD: one program, per-core input data; per-core
q layout is [full-head 1920 | half-head 960] so the program is uniform.

Host does all elementwise prep: RoPE(q), RoPE(new k), window concat,
transposes, fp16 casts.  Device does the attention proper:

  per 480-query chunk (6 per core), kk tiles of 128 in groups of 3:
     ps[b]   = K_t^T @ q_chunk      (PE, fp16 -> f32 PSUM, 6-bank region)
     ex      = exp(ps * 1/sqrt(D))  (ACT, one instr per 3-tile group, fp16 out)
     exacc  += ex[:, i, :]          (DVE fp16 adds -> per-lane denominator)
     po     += V_t^T @ ex[:, i, :]  (PE, accumulating f32 PSUM)
  DMA po (f32) and exacc (fp16) to DRAM; host computes
  out = poT / sum_lanes(exacc) and scatters.  No on-device softmax divide and
  no ones-matmul denominator pass.
"""

import numpy as np

N_CORES = 8
S = 1920
NHEADS = 12
D = 128
WIN = 5760           # attention window (keys)
KTILES = WIN // 128  # 45
QCHUNK = 480
NCHUNK = 6           # 2880 queries per core / 480
SCALE = 1.0 / np.sqrt(np.float64(D))

_PROG = None


def _rope_tables():
    """cos/sin angle tables [1920, 64] exactly as the reference builds them."""
    def rope_angles(max_len, dim, theta=10000.0):
        inv = 1.0 / (theta ** (np.arange(0, dim, 2, dtype=np.float64) / dim))
        return np.outer(np.arange(max_len, dtype=np.float64), inv)

    d = D
    freqs = np.concatenate([
        rope_angles(1024, d - 4 * (d // 6)),
        rope_angles(1024, 2 * (d // 6)),
        rope_angles(1024, 2 * (d // 6)),
    ], axis=1).astype(np.float32)          # [1024, 64]

    f, h, w = 2, 24, 40
    start_frame = 6                         # current_start // (h*w)
    c = d // 2
    s0, s1 = c - 2 * (c // 3), c // 3       # 22, 21
    ang = np.concatenate([
        np.broadcast_to(freqs[start_frame:start_frame + f, :s0][:, None, None, :], (f, h, w, s0)),
        np.broadcast_to(freqs[:h, s0:s0 + s1][None, :, None, :], (f, h, w, s1)),
        np.broadcast_to(freqs[:w, s0 + s1:][None, None, :, :], (f, h, w, s1)),
    ], axis=-1).reshape(S, c)
    return np.cos(ang).astype(np.float32), np.sin(ang).astype(np.float32)


def _rope(x, cos, sin):
    """x: [S, N, D] f32; cos/sin: [S, 64]. Interleaved-pair rotation."""
    x0, x1 = x[..., 0::2], x[..., 1::2]
    c, s = cos[:, None, :], sin[:, None, :]
    out = np.empty_like(x)
    out[..., 0::2] = x0 * c - x1 * s
    out[..., 1::2] = x0 * s + x1 * c
    return out


def _segs_for_core(c):
    """Returns ((full_head, 0, 1920), (half_head, q0, 960)) for core c."""
    if c % 2 == 0:
        return (3 * c // 2, 0, 1920), ((3 * c + 2) // 2, 0, 960)
    return ((3 * c + 2) // 2, 0, 1920), (3 * c // 2, 960, 960)


def _build_program():
    from contextlib import ExitStack
    from concourse import bacc
    import concourse.tile as tile
    import concourse.mybir as mybir

    F32 = mybir.dt.float32
    F16 = mybir.dt.float16
    EXP = mybir.ActivationFunctionType.Exp

    nc = bacc.Bacc("TRN2", target_bir_lowering=False, debug=False,
                   enable_asserts=False, num_devices=N_CORES)

    rqT = nc.dram_tensor("rqT", [128, 6 * QCHUNK], F16, kind="ExternalInput").ap()
    kT = nc.dram_tensor("kT", [2, 128, WIN], F16, kind="ExternalInput").ap()
    vin = nc.dram_tensor("vin", [2, 128, KTILES, 128], F16, kind="ExternalInput").ap()
    poT = nc.dram_tensor("poT", [NCHUNK, 128, QCHUNK], F32, kind="ExternalOutput").ap()
    exo = nc.dram_tensor("exo", [NCHUNK, 128, QCHUNK], F16, kind="ExternalOutput").ap()

    with ExitStack() as ctx:
        tc = ctx.enter_context(tile.TileContext(nc))
        inp = ctx.enter_context(tc.tile_pool(name="inp", bufs=1))
        expp = ctx.enter_context(tc.tile_pool(name="ex", bufs=4))
        accp = ctx.enter_context(tc.tile_pool(name="acc", bufs=2))
        outp = ctx.enter_context(tc.tile_pool(name="outp", bufs=2))
        pss = ctx.enter_context(tc.tile_pool(name="pss", bufs=1, space="PSUM"))
        pso = ctx.enter_context(tc.tile_pool(name="pso", bufs=2, space="PSUM"))

        q_sb = inp.tile([128, 6 * QCHUNK], F16, name="q_sb")
        k_sb = [inp.tile([128, WIN], F16, name=f"k_sb{h}") for h in range(2)]
        v_sb = [inp.tile([128, KTILES, 128], F16, name=f"v_sb{h}") for h in range(2)]
        # two 3-bank half-regions; separate tiles so the dep tracker sees the
        # ping-pong (one big tile serializes QK(g+1) behind exp(g))
        ps_h = [pss.tile([128, 3, 512], F32, name=f"ps{h}") for h in range(2)]

        # ---- input DMAs ----
        # head0's K lands in escalating pieces so QK(0) starts ASAP; V pieces
        # follow (first PV is ~2 exp-periods in); bulk q and head1 come last.
        nc.sync.dma_start(out=q_sb[:, 0:QCHUNK], in_=rqT[:, 0:QCHUNK])
        kcuts = [0, 384, 1152, 2304, 3456, 4608, WIN]
        vcuts = [0, 9, 18, 27, 36, KTILES]
        for p in range(len(kcuts) - 1):
            nc.sync.dma_start(out=k_sb[0][:, kcuts[p]:kcuts[p + 1]],
                              in_=kT[0, :, kcuts[p]:kcuts[p + 1]])
            if p >= 1:
                t0, t1 = vcuts[p - 1], vcuts[p]
                nc.sync.dma_start(out=v_sb[0][:, t0:t1, :], in_=vin[0, :, t0:t1, :])
        nc.sync.dma_start(out=v_sb[0][:, vcuts[-2]:, :], in_=vin[0, :, vcuts[-2]:, :])
        nc.sync.dma_start(out=q_sb[:, QCHUNK:], in_=rqT[:, QCHUNK:])
        for p in range(3):
            w0, w1 = p * 1920, (p + 1) * 1920
            nc.sync.dma_start(out=k_sb[1][:, w0:w1], in_=kT[1, :, w0:w1])
            t0, t1 = p * 15, (p + 1) * 15
            nc.sync.dma_start(out=v_sb[1][:, t0:t1, :], in_=vin[1, :, t0:t1, :])

        NG = KTILES // 3   # 15 groups of 3 kk-tiles
        for c in range(NCHUNK):
            hsel = 0 if c < 4 else 1
            ksb, vsb = k_sb[hsel], v_sb[hsel]
            qs = q_sb[:, c * QCHUNK:(c + 1) * QCHUNK]
            po = pso.tile([128, QCHUNK], F32, name="po")
            exacc = accp.tile([128, QCHUNK], F16, name="exacc")

            def _pv(pex, pg):
                for i in range(3):
                    t = 3 * pg + i
                    nc.tensor.matmul(out=po, lhsT=vsb[:, t, :], rhs=pex[:, i, :],
                                     start=(t == 0), stop=(t == KTILES - 1))

            # two-group PE lookahead: in-order PE must finish QK(g+1) before it
            # blocks on exp-dependent PV(g-1), or ACT starves for a full group
            pend = []     # [(ex, g)] pending PV groups, depth 2
            for g in range(NG):
                ps = ps_h[(g + c) % 2]      # 15 groups/chunk -> halves alternate
                for i in range(3):
                    t = 3 * g + i
                    nc.tensor.matmul(out=ps[:, i, 0:QCHUNK],
                                     lhsT=ksb[:, t * 128:(t + 1) * 128],
                                     rhs=qs, start=True, stop=True)
                ex = expp.tile([128, 3, QCHUNK], F16, name="ex")
                nc.scalar.activation(out=ex, in_=ps[:, 0:3, 0:QCHUNK],
                                     func=EXP, scale=float(SCALE))
                for i in range(3):
                    if g == 0 and i == 0:
                        # seed the accumulator (avoids a memset pass)
                        nc.vector.tensor_add(exacc, ex[:, 0, :], ex[:, 1, :])
                    elif g == 0 and i == 1:
                        pass
                    else:
                        nc.vector.tensor_add(exacc, exacc, ex[:, i, :])
                pend.append((ex, g))
                if len(pend) > 2:
                    _pv(*pend.pop(0))
            for ent in pend:
                _pv(*ent)
            po_sb = outp.tile([128, QCHUNK], F32, name="po_sb")
            nc.vector.tensor_copy(out=po_sb, in_=po)   # PSUM->SBUF eviction
            nc.sync.dma_start(out=poT[c], in_=po_sb)
            nc.sync.dma_start(out=exo[c], in_=exacc)

    nc.compile()
    return nc


def _get_program():
    global _PROG
    if _PROG is None:
        _PROG = _build_program()
    return _PROG


def _host_prep(q, k, v, cache_k, cache_v):
    """Build the 8 per-core input maps (rope + window concat + fp16 on host)."""
    cos, sin = _rope_tables()
    rq = _rope(np.asarray(q, np.float32)[0], cos, sin)      # [1920, 12, 128]
    rk = _rope(np.asarray(k, np.float32)[0], cos, sin)
    Kfull = np.concatenate([np.asarray(cache_k, np.float32)[0, 1920:5760], rk], axis=0)
    Vfull = np.concatenate([np.asarray(cache_v, np.float32)[0, 1920:5760],
                            np.asarray(v, np.float32)[0]], axis=0)  # [5760, 12, 128]
    rq16 = rq.astype(np.float16)
    K16 = Kfull.astype(np.float16)
    V16 = Vfull.astype(np.float16)

    in_maps = []
    for c in range(N_CORES):
        (fh, _, _), (hh, hq0, _) = _segs_for_core(c)
        qcat = np.concatenate([rq16[:, fh, :], rq16[hq0:hq0 + 960, hh, :]], axis=0)
        rqT = np.ascontiguousarray(qcat.T)                  # [128, 2880]
        kTa = np.stack([np.ascontiguousarray(K16[:, h, :].T) for h in (fh, hh)])
        va = np.stack([np.ascontiguousarray(
            V16[:, h, :].reshape(KTILES, 128, 128).transpose(1, 0, 2)) for h in (fh, hh)])
        in_maps.append({"rqT": rqT, "kT": kTa, "vin": va})
    return in_maps


def _gather(results):
    out = np.empty((1, S, NHEADS, D), np.float32)
    for c in range(N_CORES):
        poT = results[c]["poT"]                             # [6, 128, 480] f32
        exo = results[c]["exo"].astype(np.float32)          # [6, 128, 480]
        den = exo.sum(axis=1)                               # [6, 480]
        o = poT / den[:, None, :]                           # [6, 128, 480]
        (fh, _, _), (hh, hq0, _) = _segs_for_core(c)
        full = o[0:4].transpose(0, 2, 1).reshape(1920, 128)
        half = o[4:6].transpose(0, 2, 1).reshape(960, 128)
        out[0, :, fh, :] = full
        out[0, hq0:hq0 + 960, hh, :] = half
    return out


def kernel(q, k, v, cache_k, cache_v, f=2, h=24, w=40,
           current_start=5760, global_end=5760, local_end=5760, **_extra):
    from concourse.bass_utils import run_bass_kernel_spmd

    nc = _get_program()
    in_maps = _host_prep(q, k, v, cache_k, cache_v)
    res = run_bass_kernel_spmd(nc, in_maps, list(range(N_CORES)))
    return _gather(res.results)


# revision 14
# speedup vs baseline: 1.6588x; 1.0034x over previous
"""
Trainium2 Bass kernel for nn_CausalMatrixGameTransformerBlock (streaming-window
attention).

Math (shapes hardcoded from the problem spec):
  B=1, S=1920 new tokens, N=12 heads, D=128, CACHE=6720,
  f=2, h=24, w=40, current_start=global_end=local_end=5760.

  With those static ints the reference reduces to:
    rq = rope(q), rk = rope(k)
    K = concat(cache_k[:, 1920:5760], rk)   # [5760, 12, 128] window per head
    V = concat(cache_v[:, 1920:5760], v)
    out[q,n,:] = softmax(rq K^T / sqrt(128)) V    dense over 5760 keys.

Sharding: 24 units of (head, 960-query-half) -> each core gets 3 units that
cover exactly one FULL head (1920 q) plus one HALF head (960 q), so each core
loads K/V for only 2 heads.  SPMD: one program, per-core input data; per-core
q layout is [full-head 1920 | half-head 960] so the program is uniform.

Host does all elementwise prep: RoPE(q), RoPE(new k), window concat,
transposes, fp16 casts.  Device does the attention proper:

  per 480-query chunk (6 per core), kk tiles of 128 in groups of 3:
     ps[b]   = K_t^T @ q_chunk      (PE, fp16 -> f32 PSUM, 6-bank region)
     ex      = exp(ps * 1/sqrt(D))  (ACT, one instr per 3-tile group, fp16 out)
     exacc  += ex[:, i, :]          (DVE fp16 adds -> per-lane denominator)
     po     += V_t^T @ ex[:, i, :]  (PE, accumulating f32 PSUM)
  DMA po (f32) and exacc (fp16) to DRAM; host computes
  out = poT / sum_lanes(exacc) and scatters.  No on-device softmax divide and
  no ones-matmul denominator pass.
"""

import numpy as np

N_CORES = 8
S = 1920
NHEADS = 12
D = 128
WIN = 5760           # attention window (keys)
KTILES = WIN // 128  # 45
QCHUNK = 480
NCHUNK = 6           # 2880 queries per core / 480
SCALE = 1.0 / np.sqrt(np.float64(D))

_PROG = None


def _rope_tables():
    """cos/sin angle tables [1920, 64] exactly as the reference builds them."""
    def rope_angles(max_len, dim, theta=10000.0):
        inv = 1.0 / (theta ** (np.arange(0, dim, 2, dtype=np.float64) / dim))
        return np.outer(np.arange(max_len, dtype=np.float64), inv)

    d = D
    freqs = np.concatenate([
        rope_angles(1024, d - 4 * (d // 6)),
        rope_angles(1024, 2 * (d // 6)),
        rope_angles(1024, 2 * (d // 6)),
    ], axis=1).astype(np.float32)          # [1024, 64]

    f, h, w = 2, 24, 40
    start_frame = 6                         # current_start // (h*w)
    c = d // 2
    s0, s1 = c - 2 * (c // 3), c // 3       # 22, 21
    ang = np.concatenate([
        np.broadcast_to(freqs[start_frame:start_frame + f, :s0][:, None, None, :], (f, h, w, s0)),
        np.broadcast_to(freqs[:h, s0:s0 + s1][None, :, None, :], (f, h, w, s1)),
        np.broadcast_to(freqs[:w, s0 + s1:][None, None, :, :], (f, h, w, s1)),
    ], axis=-1).reshape(S, c)
    return np.cos(ang).astype(np.float32), np.sin(ang).astype(np.float32)


def _rope(x, cos, sin):
    """x: [S, N, D] f32; cos/sin: [S, 64]. Interleaved-pair rotation."""
    x0, x1 = x[..., 0::2], x[..., 1::2]
    c, s = cos[:, None, :], sin[:, None, :]
    out = np.empty_like(x)
    out[..., 0::2] = x0 * c - x1 * s
    out[..., 1::2] = x0 * s + x1 * c
    return out


def _segs_for_core(c):
    """Returns ((full_head, 0, 1920), (half_head, q0, 960)) for core c."""
    if c % 2 == 0:
        return (3 * c // 2, 0, 1920), ((3 * c + 2) // 2, 0, 960)
    return ((3 * c + 2) // 2, 0, 1920), (3 * c // 2, 960, 960)


def _build_program():
    from contextlib import ExitStack
    from concourse import bacc
    import concourse.tile as tile
    import concourse.mybir as mybir

    F32 = mybir.dt.float32
    F16 = mybir.dt.float16
    EXP = mybir.ActivationFunctionType.Exp

    nc = bacc.Bacc("TRN2", target_bir_lowering=False, debug=False,
                   enable_asserts=False, num_devices=N_CORES)

    rqT = nc.dram_tensor("rqT", [128, 6 * QCHUNK], F16, kind="ExternalInput").ap()
    kT = nc.dram_tensor("kT", [2, 128, WIN], F16, kind="ExternalInput").ap()
    vin = nc.dram_tensor("vin", [2, 128, KTILES, 128], F16, kind="ExternalInput").ap()
    poT = nc.dram_tensor("poT", [NCHUNK, 128, QCHUNK], F32, kind="ExternalOutput").ap()
    exo = nc.dram_tensor("exo", [NCHUNK, 128, QCHUNK], F16, kind="ExternalOutput").ap()

    with ExitStack() as ctx:
        tc = ctx.enter_context(tile.TileContext(nc))
        inp = ctx.enter_context(tc.tile_pool(name="inp", bufs=1))
        expp = ctx.enter_context(tc.tile_pool(name="ex", bufs=4))
        accp = ctx.enter_context(tc.tile_pool(name="acc", bufs=2))
        outp = ctx.enter_context(tc.tile_pool(name="outp", bufs=2))
        pss = ctx.enter_context(tc.tile_pool(name="pss", bufs=1, space="PSUM"))
        pso = ctx.enter_context(tc.tile_pool(name="pso", bufs=2, space="PSUM"))

        q_sb = inp.tile([128, 6 * QCHUNK], F16, name="q_sb")
        k_sb = [inp.tile([128, WIN], F16, name=f"k_sb{h}") for h in range(2)]
        v_sb = [inp.tile([128, KTILES, 128], F16, name=f"v_sb{h}") for h in range(2)]
        junk = inp.tile([128, 512], F16, name="junk")   # PE-ramp priming scratch
        # two 3-bank half-regions; separate tiles so the dep tracker sees the
        # ping-pong (one big tile serializes QK(g+1) behind exp(g))
        ps_h = [pss.tile([128, 3, 512], F32, name=f"ps{h}") for h in range(2)]

        # ---- PE ramp priming: ~3us of junk matmuls with no data deps so the
        # PE reaches max p-state right as the first real QK lands ----
        nc.vector.memset(junk, 0.0)
        for j in range(7):
            nc.tensor.matmul(out=ps_h[0][:, 0, 0:480], lhsT=junk[:, 0:128],
                             rhs=junk[:, 0:480], start=True, stop=True)

        # ---- input DMAs ----
        # head0's K lands in escalating pieces so QK(0) starts ASAP; V pieces
        # follow (first PV is ~2 exp-periods in); bulk q and head1 come last.
        # First three issue from separate DGEs (SP / DVE / Pool) in parallel.
        nc.scalar.dma_start(out=q_sb[:, 0:QCHUNK], in_=rqT[:, 0:QCHUNK])
        nc.sync.dma_start(out=k_sb[0][:, 0:384], in_=kT[0, :, 0:384])
        nc.gpsimd.dma_start(out=k_sb[0][:, 384:1152], in_=kT[0, :, 384:1152])
        kcuts = [1152, 2304, 3456, 4608, WIN]
        vcuts = [0, 9, 18, 27, 36, KTILES]
        for p in range(len(kcuts) - 1):
            nc.sync.dma_start(out=k_sb[0][:, kcuts[p]:kcuts[p + 1]],
                              in_=kT[0, :, kcuts[p]:kcuts[p + 1]])
            t0, t1 = vcuts[p], vcuts[p + 1]
            nc.sync.dma_start(out=v_sb[0][:, t0:t1, :], in_=vin[0, :, t0:t1, :])
        nc.sync.dma_start(out=v_sb[0][:, vcuts[-2]:, :], in_=vin[0, :, vcuts[-2]:, :])
        nc.sync.dma_start(out=q_sb[:, QCHUNK:], in_=rqT[:, QCHUNK:])
        for p in range(3):
            w0, w1 = p * 1920, (p + 1) * 1920
            nc.sync.dma_start(out=k_sb[1][:, w0:w1], in_=kT[1, :, w0:w1])
            t0, t1 = p * 15, (p + 1) * 15
            nc.sync.dma_start(out=v_sb[1][:, t0:t1, :], in_=vin[1, :, t0:t1, :])

        NG = KTILES // 3   # 15 groups of 3 kk-tiles
        for c in range(NCHUNK):
            hsel = 0 if c < 4 else 1
            ksb, vsb = k_sb[hsel], v_sb[hsel]
            qs = q_sb[:, c * QCHUNK:(c + 1) * QCHUNK]
            po = pso.tile([128, QCHUNK], F32, name="po")
            exacc = accp.tile([128, QCHUNK], F16, name="exacc")

            def _pv(pex, pg):
                for i in range(3):
                    t = 3 * pg + i
                    nc.tensor.matmul(out=po, lhsT=vsb[:, t, :], rhs=pex[:, i, :],
                                     start=(t == 0), stop=(t == KTILES - 1))

            # two-group PE lookahead: in-order PE must finish QK(g+1) before it
            # blocks on exp-dependent PV(g-1), or ACT starves for a full group
            pend = []     # [(ex, g)] pending PV groups, depth 2
            for g in range(NG):
                ps = ps_h[(g + c) % 2]      # 15 groups/chunk -> halves alternate
                for i in range(3):
                    t = 3 * g + i
                    nc.tensor.matmul(out=ps[:, i, 0:QCHUNK],
                                     lhsT=ksb[:, t * 128:(t + 1) * 128],
                                     rhs=qs, start=True, stop=True)
                ex = expp.tile([128, 3, QCHUNK], F16, name="ex")
                nc.scalar.activation(out=ex, in_=ps[:, 0:3, 0:QCHUNK],
                                     func=EXP, scale=float(SCALE))
                for i in range(3):
                    if g == 0 and i == 0:
                        # seed the accumulator (avoids a memset pass)
                        nc.vector.tensor_add(exacc, ex[:, 0, :], ex[:, 1, :])
                    elif g == 0 and i == 1:
                        pass
                    else:
                        nc.vector.tensor_add(exacc, exacc, ex[:, i, :])
                pend.append((ex, g))
                if len(pend) > 2:
                    _pv(*pend.pop(0))
            for ent in pend:
                _pv(*ent)
            po_sb = outp.tile([128, QCHUNK], F32, name="po_sb")
            nc.vector.tensor_copy(out=po_sb, in_=po)   # PSUM->SBUF eviction
            nc.sync.dma_start(out=poT[c], in_=po_sb)
            nc.sync.dma_start(out=exo[c], in_=exacc)

    nc.compile()
    return nc


def _get_program():
    global _PROG
    if _PROG is None:
        _PROG = _build_program()
    return _PROG


def _host_prep(q, k, v, cache_k, cache_v):
    """Build the 8 per-core input maps (rope + window concat + fp16 on host)."""
    cos, sin = _rope_tables()
    rq = _rope(np.asarray(q, np.float32)[0], cos, sin)      # [1920, 12, 128]
    rk = _rope(np.asarray(k, np.float32)[0], cos, sin)
    Kfull = np.concatenate([np.asarray(cache_k, np.float32)[0, 1920:5760], rk], axis=0)
    Vfull = np.concatenate([np.asarray(cache_v, np.float32)[0, 1920:5760],
                            np.asarray(v, np.float32)[0]], axis=0)  # [5760, 12, 128]
    rq16 = rq.astype(np.float16)
    K16 = Kfull.astype(np.float16)
    V16 = Vfull.astype(np.float16)

    in_maps = []
    for c in range(N_CORES):
        (fh, _, _), (hh, hq0, _) = _segs_for_core(c)
        qcat = np.concatenate([rq16[:, fh, :], rq16[hq0:hq0 + 960, hh, :]], axis=0)
        rqT = np.ascontiguousarray(qcat.T)                  # [128, 2880]
        kTa = np.stack([np.ascontiguousarray(K16[:, h, :].T) for h in (fh, hh)])
        va = np.stack([np.ascontiguousarray(
            V16[:, h, :].reshape(KTILES, 128, 128).transpose(1, 0, 2)) for h in (fh, hh)])
        in_maps.append({"rqT": rqT, "kT": kTa, "vin": va})
    return in_maps


def _gather(results):
    out = np.empty((1, S, NHEADS, D), np.float32)
    for c in range(N_CORES):
        poT = results[c]["poT"]                             # [6, 128, 480] f32
        exo = results[c]["exo"].astype(np.float32)          # [6, 128, 480]
        den = exo.sum(axis=1)                               # [6, 480]
        o = poT / den[:, None, :]                           # [6, 128, 480]
        (fh, _, _), (hh, hq0, _) = _segs_for_core(c)
        full = o[0:4].transpose(0, 2, 1).reshape(1920, 128)
        half = o[4:6].transpose(0, 2, 1).reshape(960, 128)
        out[0, :, fh, :] = full
        out[0, hq0:hq0 + 960, hh, :] = half
    return out


def kernel(q, k, v, cache_k, cache_v, f=2, h=24, w=40,
           current_start=5760, global_end=5760, local_end=5760, **_extra):
    from concourse.bass_utils import run_bass_kernel_spmd

    nc = _get_program()
    in_maps = _host_prep(q, k, v, cache_k, cache_v)
    res = run_bass_kernel_spmd(nc, in_maps, list(range(N_CORES)))
    return _gather(res.results)


# revision 16
# speedup vs baseline: 1.6591x; 1.0002x over previous
"""
Trainium2 Bass kernel for nn_CausalMatrixGameTransformerBlock (streaming-window
attention).

Math (shapes hardcoded from the problem spec):
  B=1, S=1920 new tokens, N=12 heads, D=128, CACHE=6720,
  f=2, h=24, w=40, current_start=global_end=local_end=5760.

  With those static ints the reference reduces to:
    rq = rope(q), rk = rope(k)
    K = concat(cache_k[:, 1920:5760], rk)   # [5760, 12, 128] window per head
    V = concat(cache_v[:, 1920:5760], v)
    out[q,n,:] = softmax(rq K^T / sqrt(128)) V    dense over 5760 keys.

Sharding: 24 units of (head, 960-query-half) -> each core gets 3 units that
cover exactly one FULL head (1920 q) plus one HALF head (960 q), so each core
loads K/V for only 2 heads.  SPMD: one program, per-core input data; per-core
q layout is [full-head 1920 | half-head 960] so the program is uniform.

Host does all elementwise prep: RoPE(q), RoPE(new k), window concat,
transposes, fp16 casts.  Device does the attention proper:

  per 480-query chunk (6 per core), kk tiles of 128 in groups of 3:
     ps[b]   = K_t^T @ q_chunk      (PE, fp16 -> f32 PSUM, 6-bank region)
     ex      = exp(ps * 1/sqrt(D))  (ACT, one instr per 3-tile group, fp16 out)
     exacc  += ex[:, i, :]          (DVE fp16 adds -> per-lane denominator)
     po     += V_t^T @ ex[:, i, :]  (PE, accumulating f32 PSUM)
  DMA po (f32) and exacc (fp16) to DRAM; host computes
  out = poT / sum_lanes(exacc) and scatters.  No on-device softmax divide and
  no ones-matmul denominator pass.
"""

import numpy as np

N_CORES = 8
S = 1920
NHEADS = 12
D = 128
WIN = 5760           # attention window (keys)
KTILES = WIN // 128  # 45
QCHUNK = 480
NCHUNK = 6           # 2880 queries per core / 480
SCALE = 1.0 / np.sqrt(np.float64(D))

_PROG = None


def _rope_tables():
    """cos/sin angle tables [1920, 64] exactly as the reference builds them."""
    def rope_angles(max_len, dim, theta=10000.0):
        inv = 1.0 / (theta ** (np.arange(0, dim, 2, dtype=np.float64) / dim))
        return np.outer(np.arange(max_len, dtype=np.float64), inv)

    d = D
    freqs = np.concatenate([
        rope_angles(1024, d - 4 * (d // 6)),
        rope_angles(1024, 2 * (d // 6)),
        rope_angles(1024, 2 * (d // 6)),
    ], axis=1).astype(np.float32)          # [1024, 64]

    f, h, w = 2, 24, 40
    start_frame = 6                         # current_start // (h*w)
    c = d // 2
    s0, s1 = c - 2 * (c // 3), c // 3       # 22, 21
    ang = np.concatenate([
        np.broadcast_to(freqs[start_frame:start_frame + f, :s0][:, None, None, :], (f, h, w, s0)),
        np.broadcast_to(freqs[:h, s0:s0 + s1][None, :, None, :], (f, h, w, s1)),
        np.broadcast_to(freqs[:w, s0 + s1:][None, None, :, :], (f, h, w, s1)),
    ], axis=-1).reshape(S, c)
    return np.cos(ang).astype(np.float32), np.sin(ang).astype(np.float32)


def _rope(x, cos, sin):
    """x: [S, N, D] f32; cos/sin: [S, 64]. Interleaved-pair rotation."""
    x0, x1 = x[..., 0::2], x[..., 1::2]
    c, s = cos[:, None, :], sin[:, None, :]
    out = np.empty_like(x)
    out[..., 0::2] = x0 * c - x1 * s
    out[..., 1::2] = x0 * s + x1 * c
    return out


def _segs_for_core(c):
    """Returns ((full_head, 0, 1920), (half_head, q0, 960)) for core c."""
    if c % 2 == 0:
        return (3 * c // 2, 0, 1920), ((3 * c + 2) // 2, 0, 960)
    return ((3 * c + 2) // 2, 0, 1920), (3 * c // 2, 960, 960)


def _build_program():
    from contextlib import ExitStack
    from concourse import bacc
    import concourse.tile as tile
    import concourse.mybir as mybir

    F32 = mybir.dt.float32
    F16 = mybir.dt.float16
    EXP = mybir.ActivationFunctionType.Exp

    nc = bacc.Bacc("TRN2", target_bir_lowering=False, debug=False,
                   enable_asserts=False, num_devices=N_CORES)

    rqT = nc.dram_tensor("rqT", [128, 6 * QCHUNK], F16, kind="ExternalInput").ap()
    kT = nc.dram_tensor("kT", [2, 128, WIN], F16, kind="ExternalInput").ap()
    vin = nc.dram_tensor("vin", [2, 128, KTILES, 128], F16, kind="ExternalInput").ap()
    poT = nc.dram_tensor("poT", [NCHUNK, 128, QCHUNK], F32, kind="ExternalOutput").ap()
    exo = nc.dram_tensor("exo", [NCHUNK, 128, QCHUNK], F16, kind="ExternalOutput").ap()

    with ExitStack() as ctx:
        tc = ctx.enter_context(tile.TileContext(nc))
        inp = ctx.enter_context(tc.tile_pool(name="inp", bufs=1))
        expp = ctx.enter_context(tc.tile_pool(name="ex", bufs=4))
        accp = ctx.enter_context(tc.tile_pool(name="acc", bufs=2))
        outp = ctx.enter_context(tc.tile_pool(name="outp", bufs=2))
        pss = ctx.enter_context(tc.tile_pool(name="pss", bufs=1, space="PSUM"))
        pso = ctx.enter_context(tc.tile_pool(name="pso", bufs=2, space="PSUM"))

        q_sb = inp.tile([128, 6 * QCHUNK], F16, name="q_sb")
        k_sb = [inp.tile([128, WIN], F16, name=f"k_sb{h}") for h in range(2)]
        v_sb = [inp.tile([128, KTILES, 128], F16, name=f"v_sb{h}") for h in range(2)]
        junk = inp.tile([128, 512], F16, name="junk")   # PE-ramp priming scratch
        # two 3-bank half-regions; separate tiles so the dep tracker sees the
        # ping-pong (one big tile serializes QK(g+1) behind exp(g))
        ps_h = [pss.tile([128, 3, 512], F32, name=f"ps{h}") for h in range(2)]

        # ---- PE ramp priming: ~3us of junk matmuls with no data deps so the
        # PE reaches max p-state right as the first real QK lands ----
        nc.gpsimd.memset(junk, 0.0)
        for j in range(7):
            nc.tensor.matmul(out=ps_h[0][:, 0, 0:512], lhsT=junk[:, 0:128],
                             rhs=junk[:, 0:512], start=True, stop=True)

        # ---- input DMAs ----
        # head0's K lands in escalating pieces so QK(0) starts ASAP; V pieces
        # follow (first PV is ~2 exp-periods in); bulk q and head1 come last.
        # First three issue from separate DGEs (SP / DVE / Pool) in parallel.
        nc.sync.dma_start(out=q_sb[:, 0:QCHUNK], in_=rqT[:, 0:QCHUNK])
        nc.sync.dma_start(out=k_sb[0][:, 0:384], in_=kT[0, :, 0:384])
        nc.gpsimd.dma_start(out=k_sb[0][:, 384:1152], in_=kT[0, :, 384:1152])
        kcuts = [1152, 2304, 3456, 4608, WIN]
        vcuts = [0, 9, 18, 27, 36, KTILES]
        for p in range(len(kcuts) - 1):
            nc.sync.dma_start(out=k_sb[0][:, kcuts[p]:kcuts[p + 1]],
                              in_=kT[0, :, kcuts[p]:kcuts[p + 1]])
            t0, t1 = vcuts[p], vcuts[p + 1]
            nc.sync.dma_start(out=v_sb[0][:, t0:t1, :], in_=vin[0, :, t0:t1, :])
        nc.sync.dma_start(out=v_sb[0][:, vcuts[-2]:, :], in_=vin[0, :, vcuts[-2]:, :])
        nc.sync.dma_start(out=q_sb[:, QCHUNK:], in_=rqT[:, QCHUNK:])
        for p in range(3):
            w0, w1 = p * 1920, (p + 1) * 1920
            nc.sync.dma_start(out=k_sb[1][:, w0:w1], in_=kT[1, :, w0:w1])
            t0, t1 = p * 15, (p + 1) * 15
            nc.sync.dma_start(out=v_sb[1][:, t0:t1, :], in_=vin[1, :, t0:t1, :])

        NG = KTILES // 3   # 15 groups of 3 kk-tiles
        for c in range(NCHUNK):
            hsel = 0 if c < 4 else 1
            ksb, vsb = k_sb[hsel], v_sb[hsel]
            qs = q_sb[:, c * QCHUNK:(c + 1) * QCHUNK]
            po = pso.tile([128, QCHUNK], F32, name="po")
            exacc = accp.tile([128, QCHUNK], F16, name="exacc")

            def _pv(pex, pg):
                for i in range(3):
                    t = 3 * pg + i
                    nc.tensor.matmul(out=po, lhsT=vsb[:, t, :], rhs=pex[:, i, :],
                                     start=(t == 0), stop=(t == KTILES - 1))

            # two-group PE lookahead: in-order PE must finish QK(g+1) before it
            # blocks on exp-dependent PV(g-1), or ACT starves for a full group
            pend = []     # [(ex, g)] pending PV groups, depth 2
            for g in range(NG):
                ps = ps_h[(g + c) % 2]      # 15 groups/chunk -> halves alternate
                for i in range(3):
                    t = 3 * g + i
                    nc.tensor.matmul(out=ps[:, i, 0:QCHUNK],
                                     lhsT=ksb[:, t * 128:(t + 1) * 128],
                                     rhs=qs, start=True, stop=True)
                ex = expp.tile([128, 3, QCHUNK], F16, name="ex")
                nc.scalar.activation(out=ex, in_=ps[:, 0:3, 0:QCHUNK],
                                     func=EXP, scale=float(SCALE))
                for i in range(3):
                    if g == 0 and i == 0:
                        # seed the accumulator (avoids a memset pass)
                        nc.vector.tensor_add(exacc, ex[:, 0, :], ex[:, 1, :])
                    elif g == 0 and i == 1:
                        pass
                    else:
                        nc.vector.tensor_add(exacc, exacc, ex[:, i, :])
                pend.append((ex, g))
                if len(pend) > 2:
                    _pv(*pend.pop(0))
            for ent in pend:
                _pv(*ent)
            po_sb = outp.tile([128, QCHUNK], F32, name="po_sb")
            nc.vector.tensor_copy(out=po_sb, in_=po)   # PSUM->SBUF eviction
            nc.sync.dma_start(out=poT[c], in_=po_sb)
            nc.sync.dma_start(out=exo[c], in_=exacc)

    nc.compile()
    return nc


def _get_program():
    global _PROG
    if _PROG is None:
        _PROG = _build_program()
    return _PROG


def _host_prep(q, k, v, cache_k, cache_v):
    """Build the 8 per-core input maps (rope + window concat + fp16 on host)."""
    cos, sin = _rope_tables()
    rq = _rope(np.asarray(q, np.float32)[0], cos, sin)      # [1920, 12, 128]
    rk = _rope(np.asarray(k, np.float32)[0], cos, sin)
    Kfull = np.concatenate([np.asarray(cache_k, np.float32)[0, 1920:5760], rk], axis=0)
    Vfull = np.concatenate([np.asarray(cache_v, np.float32)[0, 1920:5760],
                            np.asarray(v, np.float32)[0]], axis=0)  # [5760, 12, 128]
    rq16 = rq.astype(np.float16)
    K16 = Kfull.astype(np.float16)
    V16 = Vfull.astype(np.float16)

    in_maps = []
    for c in range(N_CORES):
        (fh, _, _), (hh, hq0, _) = _segs_for_core(c)
        qcat = np.concatenate([rq16[:, fh, :], rq16[hq0:hq0 + 960, hh, :]], axis=0)
        rqT = np.ascontiguousarray(qcat.T)                  # [128, 2880]
        kTa = np.stack([np.ascontiguousarray(K16[:, h, :].T) for h in (fh, hh)])
        va = np.stack([np.ascontiguousarray(
            V16[:, h, :].reshape(KTILES, 128, 128).transpose(1, 0, 2)) for h in (fh, hh)])
        in_maps.append({"rqT": rqT, "kT": kTa, "vin": va})
    return in_maps


def _gather(results):
    out = np.empty((1, S, NHEADS, D), np.float32)
    for c in range(N_CORES):
        poT = results[c]["poT"]                             # [6, 128, 480] f32
        exo = results[c]["exo"].astype(np.float32)          # [6, 128, 480]
        den = exo.sum(axis=1)                               # [6, 480]
        o = poT / den[:, None, :]                           # [6, 128, 480]
        (fh, _, _), (hh, hq0, _) = _segs_for_core(c)
        full = o[0:4].transpose(0, 2, 1).reshape(1920, 128)
        half = o[4:6].transpose(0, 2, 1).reshape(960, 128)
        out[0, :, fh, :] = full
        out[0, hq0:hq0 + 960, hh, :] = half
    return out


def kernel(q, k, v, cache_k, cache_v, f=2, h=24, w=40,
           current_start=5760, global_end=5760, local_end=5760, **_extra):
    from concourse.bass_utils import run_bass_kernel_spmd

    nc = _get_program()
    in_maps = _host_prep(q, k, v, cache_k, cache_v)
    res = run_bass_kernel_spmd(nc, in_maps, list(range(N_CORES)))
    return _gather(res.results)
